# revision 5
# baseline (speedup 1.0000x reference)
"""Trainium2 Bass kernel (v14: + staged arena copy off the psum recycle path) for nn_BlendedMLP: 7 tiny MLPs (1->16->16->1, tanh)
blended by cubic B-spline basis weights, batch 4M, data-parallel over 8 cores.

v2: fp16 tensor-engine path (4x fp32 rate) + operand-swapped MM3.

Stream A per chunk-PAIR (1024 elements, hidden-on-partitions):
  MM1  2x[K=3 (ones,x_hi,x_lo) -> M=112]  h1pre = W1*(x_hi+x_lo) + b1   (PE fp16)
  tanh [112,1024]                          h1 (fp16)                     (ACT)
  MM2  2x[K=112 -> M=112]                  h2pre = blockdiag(W2) @ h1    (PE fp16)
  tanh(+b2) [112,1024]                     h2x (fp16, ones row 112)      (ACT)
  MM3-swap: per 128-elem block bi (8/pair): stationary = h2x block
    [113,128], moving = w3cat [113,40] = [A_hi | A_lo] fp16 split of the
    banded blend matrices; out ggT[128, 40bi:+40] PSUM = per-element
    coefficients ALREADY in element-on-partition layout (no PE transpose).
  DVE: arena = ggT_hi + ggT_lo (copy + add, [128,160]/pair).
Stream B (batch-on-partitions, [128, W] elementwise, fp32) unchanged:
  out(x) = sum_m relu(10x-m)^3 * g_m  (x<0.5)  mirrored for x>=0.5.

fp16 error budget: W/h fp16 rounding enters as an error on the MLP outputs
y_i, which reaches the result as sum_i B_i(x) dy_i <= max|dy| ~1e-3 (no
amplification: sum_m relu^3 G[m,i] == B_i exactly). The amplified path
(per-(m,i) weight rounding in MM3) is killed by the A_hi+A_lo split.
x is split x_hi+x_lo (both fp16) so x enters exactly to ~1e-7.
"""

import sys

for _p in ("/opt/trn_rl_repo",):
    if _p not in sys.path:
        sys.path.insert(0, _p)

import numpy as np
from contextlib import ExitStack
from math import comb

import concourse.bass as bass
import concourse.bacc as bacc
import concourse.tile as tile
from concourse.tile import add_dep_helper
from concourse import mybir
from concourse.bass_utils import run_bass_kernel_spmd

# ---------------- problem constants (hardcoded per contract) ----------------
N_MLP, H, R = 7, 16, 112          # experts, hidden, R = 7*16
BATCH = 4_000_000
NCORES = 8
PER = BATCH // NCORES             # 500_000 per core
C = 992                           # chunks of 512 per core (padded)
F = 4 * C                         # 3968 stream-B cols; 128*F = 507_904 padded
CB = 124                          # chunks per stream-B super-block (8 blocks)
NSB = C // CB
W = 4 * CB                        # 496 stream-B cols per super-block
NG = C // 32                      # xrow DMA groups (32 chunks = 16 pairs each)
NPAIR = C // 2                    # 496 pairs
NPAD = 128 * F
PAD_VAL = 0.25

FP = mybir.dt.float32
F16 = mybir.dt.float16
ALU = mybir.AluOpType
AF = mybir.ActivationFunctionType


# ---------------- host-side constant packing ----------------
def _build_consts(W1, b1, W2, b2, W3, b3):
    W1 = np.asarray(W1, np.float64)
    b1 = np.asarray(b1, np.float64)
    W2 = np.asarray(W2, np.float64)
    b2 = np.asarray(b2, np.float64)
    W3 = np.asarray(W3, np.float64)
    b3 = np.asarray(b3, np.float64)

    # banded blend matrices [10, 7]
    G_R = np.zeros((10, 7))
    G_L = np.zeros((10, 7))
    for m in range(10):
        for i in range(7):
            k = m - i
            if 0 <= k <= 4:
                G_R[m, i] = (-1) ** k * comb(4, k) / 6.0
    for mm in range(1, 11):
        for i in range(7):
            k = i + 4 - mm
            if 0 <= k <= 4:
                G_L[mm - 1, i] = (-1) ** k * comb(4, k) / 6.0

    # w1cat fp16 [99, 112]: rows 32r+0 = b1 (ones row coeffs),
    # 32r+1 = W1 (x_hi), 32r+2 = W1 (x_lo); replicated across the 4 array
    # row-groups so MM1 can issue from any group.
    w1cat = np.zeros((99, 128))
    for rr in range(4):
        w1cat[32 * rr + 0, :R] = b1.reshape(R)
        w1cat[32 * rr + 1, :R] = W1.reshape(R)
        w1cat[32 * rr + 2, :R] = W1.reshape(R)

    # w2cat [113,128]: blockdiag(W2^T) cols 0..111, row 112 = b2 (ones fold);
    # padded to 128 weight columns so FWL (fast weight load) kicks in.
    w2t = np.zeros((R + 1, 128))
    for n in range(N_MLP):
        w2t[16 * n:16 * n + H, 16 * n:16 * n + H] = W2[n].T
    w2t[R, :R] = b2.reshape(R)

    # A = [10,112] banded blend x W3row; w3cat [113, 40] fp16 hi/lo split
    W3row = np.zeros((7, R))
    for i in range(N_MLP):
        W3row[i, 16 * i:16 * i + H] = W3[i, 0, :]
    A_R = G_R @ W3row             # [10, 112]
    A_L = G_L @ W3row
    b3f = b3.reshape(7)
    # only 5 terms per side survive the mask: right m=0..4 (x<0.5),
    # left mm=10..6 (x>=0.5).  col 5+k = left term mm=10-k.
    gb_R = G_R @ b3f
    gb_L = G_L @ b3f
    w3g = np.zeros((R + 1, 10))
    w3g[:R, 0:5] = A_R.T[:, 0:5]
    w3g[R, 0:5] = gb_R[0:5]
    for k in range(5):
        w3g[:R, 5 + k] = A_L.T[:, 9 - k]
        w3g[R, 5 + k] = gb_L[9 - k]
    w3hi = w3g.astype(np.float16)
    w3lo = (w3g - w3hi.astype(np.float64)).astype(np.float16)
    w3cat = np.concatenate([w3hi, w3lo], axis=1)  # [113, 20]

    return {
        "w1cat": w1cat.astype(np.float16),
        "w2t": w2t.astype(np.float16),
        "w3cat": w3cat,
    }


LIMIT_SB = None   # for timing experiments: emit only this many super-blocks


# ---------------- device program ----------------
def _build_nc():
    nc = bacc.Bacc()
    d_xrows = nc.declare_dram_parameter("xrows", [NG, 4, 3, 4096], F16,
                                        isOutput=False)
    d_xb = nc.declare_dram_parameter("xb", [128, F], FP, isOutput=False)
    d_w1 = nc.declare_dram_parameter("w1cat", [99, 128], F16, isOutput=False)
    d_w2 = nc.declare_dram_parameter("w2t", [R + 1, 128], F16, isOutput=False)
    d_w3 = nc.declare_dram_parameter("w3cat", [R + 1, 20], F16, isOutput=False)
    d_out = nc.declare_dram_parameter("out", [128, F], FP, isOutput=True)

    with tile.TileContext(nc) as tc, ExitStack() as ctx:
        singles = ctx.enter_context(tc.tile_pool(name="singles", bufs=1))
        sb_arena = ctx.enter_context(tc.tile_pool(name="sb_arena", bufs=2))
        sb_tmp = ctx.enter_context(tc.tile_pool(name="sb_tmp", bufs=2))
        sb_tmp1 = ctx.enter_context(tc.tile_pool(name="sb_tmp1", bufs=1))
        sb_stage = ctx.enter_context(tc.tile_pool(name="sb_stage", bufs=2))
        ps_h1 = ctx.enter_context(tc.tile_pool(name="ps_h1", bufs=2, space="PSUM"))
        ps_h2 = ctx.enter_context(tc.tile_pool(name="ps_h2", bufs=1, space="PSUM"))
        ps_g = ctx.enter_context(tc.tile_pool(name="ps_g", bufs=2, space="PSUM"))

        # constants: DMA into staging, then DVE-copy into the tiles PE reads,
        # so every PE-side init dependency collapses onto the DVE semaphore.
        w1l = singles.tile([99, 128], F16)
        nc.sync.dma_start(out=w1l, in_=d_w1[:, :])
        w2l = singles.tile([R + 1, 128], F16)
        nc.sync.dma_start(out=w2l, in_=d_w2[:, :])
        w3l = singles.tile([R + 1, 20], F16)
        nc.sync.dma_start(out=w3l, in_=d_w3[:, :])
        w1s = singles.tile([99, 128], F16)
        nc.vector.tensor_copy(w1s, w1l)
        w2s = singles.tile([R + 1, 128], F16)
        nc.vector.tensor_copy(w2s, w2l)
        w3s = singles.tile([R + 1, 20], F16)
        nc.vector.tensor_copy(w3s, w3l)

        # xrb 3-ring per 16-pair group; ones rows arrive with the DMA.
        xrb = [singles.tile([99, 4096], F16, name=f"xrb{i}") for i in range(3)]
        # prologue: groups 0+1 land before the big xb DMA queues behind
        for gg in range(2):
            for r4 in range(4):
                nc.sync.dma_start(
                    out=xrb[gg][32 * r4:32 * r4 + 3, :],
                    in_=d_xrows[gg, r4, :, :],
                )
        # resident stream-B input / output (after xrb prologue in DMA queue)
        xbs = singles.tile([128, F], FP)
        nc.sync.dma_start(out=xbs, in_=d_xb[:, :])
        outs = singles.tile([128, F], FP)

        # h1/h2x ping-pong [128, 1024]; rows 112.. memset 1.0 (row 112 = ones)
        h1e = [singles.tile([128, 2048], F16, name=f"h1e{i}") for i in range(2)]
        h2e = [singles.tile([128, 2048], F16, name=f"h2e{i}") for i in range(2)]
        for t in h1e + h2e:
            # rows 96..111 are rewritten by tanh every pair; row 112 stays 1.0
            nc.vector.memset(t[96:128, :], 1.0)

        tp_hist = []
        def gen_streamB(s, arena_tile, w0, w1, tg):
            # yields after each emitted instruction so the caller can
            # interleave these DVE ops with the next super-block's pair ops
            WS = w1 - w0
            xs = xbs[:, W * s + w0:W * s + w1]
            av = arena_tile.rearrange("p (w j) -> p w j", j=10)[:, w0:w1, :]
            mask = sb_tmp1.tile([128, WS], FP, tag="mask" + tg)
            nc.vector.tensor_scalar(mask, xs, 0.5, None, ALU.is_ge)
            yield
            acc_r = None
            u = sb_tmp.tile([128, WS], FP, tag="u" + tg)
            nc.vector.tensor_scalar(u, xs, 10.0, 0.0, ALU.mult, ALU.add)
            yield
            for m in range(5):
                if m > 0:
                    u2 = sb_tmp.tile([128, WS], FP, tag="u" + tg)
                    nc.vector.tensor_scalar(u2, u, 1.0, 0.0, ALU.subtract,
                                            ALU.max)
                    u = u2
                    yield
                sq = sb_tmp1.tile([128, WS], FP, tag="sq" + tg)
                nc.vector.tensor_tensor(sq, u, u, ALU.mult)
                yield
                p = sb_tmp1.tile([128, WS], FP, tag="p" + tg)
                nc.vector.tensor_tensor(p, u, av[:, :, m], ALU.mult)
                yield
                if acc_r is None:
                    acc_r = sb_tmp.tile([128, WS], FP, tag="acc" + tg)
                    nc.vector.tensor_tensor(acc_r, sq, p, ALU.mult)
                    yield
                else:
                    q_ = sb_tmp1.tile([128, WS], FP, tag="q" + tg)
                    nc.vector.tensor_tensor(q_, sq, p, ALU.mult)
                    yield
                    acc2 = sb_tmp.tile([128, WS], FP, tag="acc" + tg)
                    nc.vector.tensor_tensor(acc2, acc_r, q_, ALU.add)
                    acc_r = acc2
                    yield
            acc_l = None
            v = sb_tmp.tile([128, WS], FP, tag="v" + tg)
            nc.vector.tensor_scalar(v, xs, -10.0, 10.0, ALU.mult, ALU.add)
            yield
            for mm in range(10, 5, -1):
                if mm < 10:
                    v2 = sb_tmp.tile([128, WS], FP, tag="v" + tg)
                    nc.vector.tensor_scalar(v2, v, 1.0, 0.0, ALU.subtract,
                                            ALU.max)
                    v = v2
                    yield
                sq = sb_tmp1.tile([128, WS], FP, tag="sq" + tg)
                nc.vector.tensor_tensor(sq, v, v, ALU.mult)
                yield
                p = sb_tmp1.tile([128, WS], FP, tag="p" + tg)
                nc.vector.tensor_tensor(p, v, av[:, :, 5 + (10 - mm)], ALU.mult)
                yield
                if acc_l is None:
                    acc_l = sb_tmp.tile([128, WS], FP, tag="accl" + tg)
                    nc.vector.tensor_tensor(acc_l, sq, p, ALU.mult)
                    yield
                else:
                    q_ = sb_tmp1.tile([128, WS], FP, tag="q" + tg)
                    nc.vector.tensor_tensor(q_, sq, p, ALU.mult)
                    yield
                    acc2 = sb_tmp.tile([128, WS], FP, tag="accl" + tg)
                    nc.vector.tensor_tensor(acc2, acc_l, q_, ALU.add)
                    acc_l = acc2
                    yield
            d = sb_tmp1.tile([128, WS], FP, tag="d" + tg)
            nc.vector.tensor_tensor(d, acc_l, acc_r, ALU.subtract)
            yield
            md = sb_tmp1.tile([128, WS], FP, tag="md" + tg)
            nc.vector.tensor_tensor(md, mask, d, ALU.mult)
            yield
            nc.vector.tensor_tensor(outs[:, W * s + w0:W * s + w1], acc_r, md,
                                    ALU.add)
            yield

        pending = None


        for s in range(NSB if LIMIT_SB is None else LIMIT_SB):
            arena_tile = sb_arena.tile([128, 40 * CB], FP, tag="arena")
            for cl4 in range(0, CB, 4):
                c0 = s * CB + cl4        # first chunk of the quad
                q = c0 // 4              # global quad index
                g = c0 // 32
                xr = xrb[g % 3]
                if c0 % 32 == 0 and g + 1 < NG:
                    # prefetch the NEXT group one group (8 quads) ahead
                    xrn = xrb[(g + 1) % 3]
                    for r4 in range(4):
                        nc.sync.dma_start(
                            out=xrn[32 * r4:32 * r4 + 3, :],
                            in_=d_xrows[g + 1, r4, :, :],
                        )

                quad = ps_h1.tile([128, 2048], FP, tag="quad")
                for k in range(4):
                    cc = c0 + k
                    rr, hh = cc % 4, (cc % 32) // 4
                    mm1 = nc.tensor.matmul(
                        quad[0:128, 512 * k:512 * k + 512],
                        w1s[32 * rr:32 * rr + 3, 0:128],
                        xr[32 * rr:32 * rr + 3, 512 * hh:512 * hh + 512],
                        start=True, stop=True,
                        tile_position=(32 * rr, 0),
                    )
                    if len(tp_hist) >= 2:
                        add_dep_helper(mm1.ins, tp_hist[-2].ins, False,
                                       "pe order")
                    tp_hist.append(mm1)
                h1 = h1e[q % 2]
                nc.scalar.activation(h1[0:R, :], quad[0:R, :], AF.Tanh)

                for k in range(4):
                    nc.tensor.matmul(
                        quad[0:128, 512 * k:512 * k + 512],
                        w2s[0:R + 1, 0:128],
                        h1[0:R + 1, 512 * k:512 * k + 512],
                        start=True, stop=True,
                    )
                h2x = h2e[q % 2]
                nc.scalar.activation(h2x[0:R, :], quad[0:R, :], AF.Tanh)

                # MM3-swap: 16 blocks of 128 elements into quad[:, 0:320]
                # (dead after tanh2); coefficients land pre-transposed.
                for bi in range(16):
                    nc.tensor.matmul(
                        quad[0:128, 20 * bi:20 * bi + 20],
                        h2x[0:R + 1, 128 * bi:128 * bi + 128],
                        w3s[0:R + 1, 0:20],
                        start=True, stop=True,
                    )
                # one short PSUM->SBUF copy frees the quad tile (recycle
                # path for MM1(q+2)); the hi+lo add runs later from SBUF.
                stage = sb_stage.tile([128, 320], FP, tag="stage")
                nc.vector.tensor_copy(stage, quad[0:128, 0:320])
                gv = stage.rearrange("p (b k) -> p b k", k=20)
                dst = arena_tile[:, 40 * cl4:40 * cl4 + 160].rearrange(
                    "p (b j) -> p b j", j=10
                )
                nc.vector.tensor_tensor(dst, gv[:, :, 0:10], gv[:, :, 10:20],
                                        ALU.add)
                if (s == (NSB if LIMIT_SB is None else LIMIT_SB) - 1
                        and cl4 in (32, 64, 96) and pending is not None):
                    # last SB: drain, then start the next ready quarter
                    for _ in pending:
                        pass
                    kq = cl4 // 32 - 1
                    pending = gen_streamB(s, arena_tile, kq * (W // 4),
                                          (kq + 1) * (W // 4), "q")
                if pending is not None:
                    next(pending, None)
                    next(pending, None)
                    next(pending, None)
                    next(pending, None)

            if pending is not None:
                for _ in pending:
                    pass
            last = (s == (NSB if LIMIT_SB is None else LIMIT_SB) - 1)
            if not last:
                pending = gen_streamB(s, arena_tile, 0, W, "")
            else:
                pending_tail = gen_streamB(s, arena_tile, 3 * (W // 4), W, "q")
        for _ in pending:
            pass
        for _ in pending_tail:
            pass

        nc.sync.dma_start(out=d_out[:, :], in_=outs)

    nc.compile()
    return nc


_NC_CACHE = None


def _get_nc():
    global _NC_CACHE
    if _NC_CACHE is None:
        _NC_CACHE = _build_nc()
    return _NC_CACHE


def _pack_core_inputs(xp):
    """xp: padded per-core [NPAD] -> {'xb', 'xrows'} device layouts."""
    xb = xp.reshape(128, F)
    # chunk rows: row[c, 128t+pp] = xp[pp*F + 4c + t]
    rows = np.ascontiguousarray(xp.reshape(128, C, 4).transpose(1, 2, 0)).reshape(
        C, 512
    )
    # groups of 32 chunks; chunk cc -> row-group r=cc%4, col 512*((cc%32)//4)
    pr = np.ascontiguousarray(
        rows.reshape(NG, 8, 4, 512).transpose(0, 2, 1, 3)
    ).reshape(NG, 4, 4096)
    hi = pr.astype(np.float16)
    lo = (pr - hi.astype(np.float32)).astype(np.float16)
    ones = np.ones_like(hi)
    xr = np.ascontiguousarray(
        np.stack([ones, hi, lo], axis=2)
    )  # [NG, 4, 3, 4096]
    return {"xb": xb, "xrows": xr}


def kernel(x, knots, W1, b1, W2, b2, W3, b3, **_unused):
    x = np.asarray(x, np.float32)
    consts = _build_consts(W1, b1, W2, b2, W3, b3)
    nc = _get_nc()

    xf = x.reshape(-1)
    in_maps = []
    for ci in range(NCORES):
        xc = xf[ci * PER:(ci + 1) * PER]
        xp = np.full(NPAD, PAD_VAL, np.float32)
        xp[:PER] = xc
        m = _pack_core_inputs(xp)
        m.update(consts)
        in_maps.append(m)

    res = run_bass_kernel_spmd(nc, in_maps, list(range(NCORES)))
    out = np.empty((BATCH,), np.float32)
    for ci in range(NCORES):
        out[ci * PER:(ci + 1) * PER] = res.results[ci]["out"].reshape(-1)[:PER]
    return out.reshape(BATCH, 1)


if __name__ == "__main__":
    _get_nc()
    print("nc built ok")
